# revision 49
# baseline (speedup 1.0000x reference)
"""Bass/Trainium2 kernel for nn_Attention (ragged masked-softmax attention).

Math (per batch b with valid length L):
    c_b      = W_h @ hidden[:, b] + b_attn                  # [2H], W_h = W_attn[:, :H]
    e[s, :]  = tanh(W_e @ x_s + c_b)                        # W_e = W_attn[:, H:]
    score[s] = w_v . e[s, :] + b_v            (s < L)
    energy   = softmax(score[:L]);  context = energy @ X[:L]

Device strategy: the ragged work is split into fixed 256-position chunks
("units", 72 total for the graded lengths), distributed evenly over 8 cores
(one identical static SPMD program).  Each unit produces flash-softmax
partials (m, Z, ctx) which the host merges exactly.

The dominant e-matmul runs in fp8e4m3 with DoubleRow perf mode (2x the fp16
PE rate).  The fp8 quantization noise n on z = W_e x feeds the scores as
wv.(tanh'(z+c) (.) n); it is suppressed by an exact rank-1 correction
    score = wv.t~  -  a*(wv.z~ - wv.z),      a ~= E[tanh'] = 0.6
where BOTH linear functionals are computed exactly on the host (free):
wv.z = (W_e^T wv).x over the exact inputs, and wv.z~ = (W8^T wv).X8 over
the quantized operands the PE actually multiplies.  Their difference is
folded into the per-position mask/base row, so the device runs nothing
but the pure fp8 matmul + tanh + scores + softmax + context.  The exp
skips the max-subtraction pass (scores are provably bounded, so fp32
exp cannot overflow; the host flash-merge uses m=0).  Measured rel-err
~1.0e-2 against the fp32 reference (gate 2e-2).
"""

import numpy as np
import ml_dtypes

import concourse.bass as bass
import concourse.mybir as mybir
import concourse.tile as tile
from concourse import bacc
from concourse.bass_utils import run_bass_kernel_spmd

B, S, H = 16, 2048, 1024
H2 = 2 * H            # 2048 output features / encoder dim
CHUNK = 256           # sequence positions per work unit
N_CORES = 8
FB = H2 // 128        # 16 f-blocks of the contraction dim (encoder features)
OB = H2 // 128        # 16 o-blocks of the output features
NEG = -30000.0        # masked-score offset (exp underflows to exactly 0)
ALPHA = 0.6           # tanh'-projection coefficient of the fp8 correction
SW = 256.0            # fp8 scale on W_e
SX = 16.0             # fp8 scale on X
SV = 1024.0           # fp8 scale on the correction rows v = W8^T wv
INV_SWSX = 1.0 / (SW * SX)
CH = -ALPHA / (SV * SX)   # fold scale of the correction rows

F8 = mybir.dt.float8e4
F16 = mybir.dt.float16
F32 = mybir.dt.float32
NP8 = ml_dtypes.float8_e4m3


def build_program(nchunk: int):
    nc = bacc.Bacc()

    xt_ext = nc.declare_dram_parameter("xt", [nchunk, 128, FB, CHUNK], F8, isOutput=False)
    xn_ext = nc.declare_dram_parameter("xn", [nchunk, 128, CHUNK // 128, H2], F16, isOutput=False)
    mask_ext = nc.declare_dram_parameter("mask", [nchunk, CHUNK], F32, isOutput=False)
    c_ext = nc.declare_dram_parameter("cb", [128, OB, nchunk], F32, isOutput=False)
    wet_ext = nc.declare_dram_parameter("wet", [OB, 128, FB, 128], F8, isOutput=False)
    wv_ext = nc.declare_dram_parameter("wv", [128, OB], F16, isOutput=False)
    ctx_out = nc.declare_dram_parameter("out_ctx", [nchunk, H2], F32, isOutput=True)
    mz_out = nc.declare_dram_parameter("out_mz", [nchunk, 2], F32, isOutput=True)

    SB = CHUNK // 128   # s-blocks per unit for the context matmul
    DQ = H2 // 512      # 512-wide output quarters for the context matmul

    from contextlib import ExitStack
    with tile.TileContext(nc) as tc, ExitStack() as stk:
        singles = stk.enter_context(tc.tile_pool(name="singles", bufs=1))
        xtp = stk.enter_context(tc.tile_pool(name="xtp", bufs=3))
        xnp = stk.enter_context(tc.tile_pool(name="xnp", bufs=3))
        tp = stk.enter_context(tc.tile_pool(name="tp", bufs=2))
        smalls = stk.enter_context(tc.tile_pool(name="smalls", bufs=3))
        eps = stk.enter_context(tc.tile_pool(name="eps", bufs=4, space="PSUM"))
        sps = stk.enter_context(tc.tile_pool(name="sps", bufs=2, space="PSUM"))
        cps = stk.enter_context(tc.tile_pool(name="cps", bufs=1, space="PSUM"))

        # resident weights as one tile per o-block (fine-grained DMA deps so
        # the PE can start as soon as the first o-block's weights land)
        wet_sb = []
        wv_sb = singles.tile([128, OB], F16)
        c_sb = singles.tile([128, OB, nchunk], F32)
        mask_sb = singles.tile([1, nchunk, CHUNK], F32)
        xt0_sb = xtp.tile([128, FB, CHUNK], F8, tag="xt")
        xt1_sb = None
        if nchunk >= 2:
            xt1_sb = xtp.tile([128, FB, CHUNK], F8, tag="xt", name="xt1_sb")
        for ob in range(OB):
            if ob == 0:
                nc.sync.dma_start(out=xt0_sb[:], in_=xt_ext[0])
            if ob == 1 and xt1_sb is not None:
                nc.sync.dma_start(out=xt1_sb[:], in_=xt_ext[1])
            w1 = singles.tile([128, FB, 128], F8, tag=f"wet{ob}")
            nc.sync.dma_start(out=w1[:], in_=wet_ext[ob])
            wet_sb.append(w1)
            if ob == 0:
                nc.sync.dma_start(out=c_sb[:], in_=c_ext[:])
                nc.sync.dma_start(out=wv_sb[:], in_=wv_ext[:])
                nc.sync.dma_start(out=mask_sb[0:1, :, :], in_=mask_ext[:])
        mz_all = singles.tile([1, nchunk, 2], F32)
        nc.vector.memset(mz_all[:], 0.0)
        ident_sb = singles.tile([1, 1], F16)
        nc.vector.memset(ident_sb[:], 1.0)

        def emit_xn_dma(p):
            i, xn_sb = p[0], p[2]
            nc.sync.dma_start(out=xn_sb[:], in_=xn_ext[i])

        def emit_ctx(p):
            # 4 output quarters on the 4 PE column groups, running concurrently
            i, pt_sb, xn_sb = p[0], p[1], p[2]
            ctx_sb = smalls.tile([1, H2], F32, tag="ctx")
            ctx_ps = cps.tile([128, 512], F32, tag="cps")
            for dq in range(DQ):
                for sb in range(SB):
                    nc.tensor.matmul(
                        ctx_ps[32 * dq:32 * dq + 1, :],
                        lhsT=pt_sb[:, sb:sb + 1],
                        rhs=xn_sb[:, sb, dq * 512:(dq + 1) * 512],
                        start=(sb == 0), stop=(sb == SB - 1),
                        tile_position=(0, 32 * dq),
                    )
            for dq in range(DQ):
                nc.scalar.copy(
                    out=ctx_sb[0:1, dq * 512:(dq + 1) * 512],
                    in_=ctx_ps[32 * dq:32 * dq + 1, :])
            nc.sync.dma_start(out=ctx_out[i], in_=ctx_sb[0:1, :])

        def emit_egroup(i0, nw, xt_sb, t_sb, ob):
            e_ps = eps.tile([128, CHUNK], F32, tag="e")
            for fb in range(0, FB, 2):
                nc.tensor.matmul(
                    e_ps[:],
                    lhsT=wet_sb[ob][:, fb:fb + 2, :],
                    rhs=xt_sb[:, fb:fb + 2, :],
                    start=(fb == 0), stop=(fb == FB - 2),
                    perf_mode=mybir.MatmulPerfMode.DoubleRow,
                )
            nc.scalar.activation(
                out=t_sb[:, ob, :], in_=e_ps[:],
                func=mybir.ActivationFunctionType.Tanh,
                bias=c_sb[:, ob, i0:i0 + 1], scale=INV_SWSX,
            )

        def emit_scores(t_sb):
            # scores[s] = sum_o wv[o] t[o, s] -> 4 partial rows on the 4 PE
            # column groups running concurrently
            s_ps = sps.tile([128, CHUNK], F32, tag="st", bufs=1)
            for r in range(OB // 4):
                for j in range(4):
                    ob = r * 4 + j
                    nc.tensor.matmul(
                        s_ps[32 * j:32 * j + 1, :],
                        lhsT=wv_sb[:, ob:ob + 1],
                        rhs=t_sb[:, ob, :],
                        start=(r == 0), stop=(r == OB // 4 - 1),
                        tile_position=(0, 32 * j),
                    )
            return s_ps

        def emit_softmax(i, s_ps):
            # masked softmax partials: fold the 4 score rows + mask/base row
            # (DVE may read at most one PSUM operand per op -> serial chain)
            acc_sb = []
            srcs = [s_ps[32 * j:32 * j + 1, :] for j in range(4)]
            for j, src in enumerate(srcs):
                prev = mask_sb[0:1, i, :] if j == 0 else acc_sb[-1][:]
                a = smalls.tile([1, CHUNK], F32, tag=f"fold{j}")
                nc.vector.tensor_tensor(
                    out=a[:], in0=src, in1=prev,
                    op=mybir.AluOpType.add,
                )
                acc_sb.append(a)
            sc_sb = acc_sb[-1]
            # scores are bounded (|s| < ~10) -> exp never overflows fp32, so
            # the max-subtraction pass is skipped entirely (host merge m=0)
            p_sb = smalls.tile([1, CHUNK], F16, tag="p")
            z_sb = smalls.tile([1, 1], F32, tag="z")
            nc.scalar.activation(
                out=p_sb[:], in_=sc_sb[:],
                func=mybir.ActivationFunctionType.Exp,
                bias=0.0, scale=1.0, accum_out=z_sb[:],
            )
            nc.gpsimd.tensor_copy(out=mz_all[0:1, i, 1:2], in_=z_sb[:])
            xn_sb = xnp.tile([128, SB, H2], F16, tag="xn")
            return [i, p_sb, xn_sb]

        def emit_pt(p):
            # p row -> column layout [128, SB] via PE transpose.  Deferred to
            # the NEXT unit's PE stream (after its e-groups) so the transpose
            # never waits on the softmax chain.
            i, p_sb, xn_sb = p
            pt_sb = smalls.tile([128, SB], F16, tag="pt")
            for sb in range(SB):
                t_ps = sps.tile([128, 1], F16, tag="tp", bufs=2)
                nc.tensor.transpose(
                    t_ps[:], p_sb[0:1, sb * 128:(sb + 1) * 128], ident_sb[:])
                nc.vector.tensor_copy(out=pt_sb[:, sb:sb + 1], in_=t_ps[:])
            p[1] = pt_sb

        pending = []
        if nchunk >= 2:
            # ob-interleaved warm-up of units 0+1: each weight tile arriving
            # on the ramping DMA feeds TWO e-groups, so the PE stays busy
            # while the weight stream is still behind
            t0_sb = tp.tile([128, OB, CHUNK], F16, tag="t")
            t1_sb = tp.tile([128, OB, CHUNK], F16, tag="t")
            for ob in range(8):
                emit_egroup(0, 1, xt0_sb, t0_sb, ob)
            for ob in range(8, OB):
                emit_egroup(1, 1, xt1_sb, t1_sb, ob - 8)
                emit_egroup(0, 1, xt0_sb, t0_sb, ob)
            for ob in range(OB - 8, OB):
                emit_egroup(1, 1, xt1_sb, t1_sb, ob)
            s_ps = emit_scores(t0_sb)
            pending.append(emit_softmax(0, s_ps))
            s_ps = emit_scores(t1_sb)
            pending.append(emit_softmax(1, s_ps))
            start_i = 2
        else:
            start_i = 0
        for i0 in range(start_i, nchunk):
            if i0 == 0:
                xt_sb = xt0_sb
            else:
                xt_sb = xtp.tile([128, FB, CHUNK], F8, tag="xt")
                nc.sync.dma_start(out=xt_sb[:], in_=xt_ext[i0])
            for p in pending:
                emit_xn_dma(p)  # queued behind this unit's xt

            t_sb = tp.tile([128, OB, CHUNK], F16, tag="t")
            for ob in range(OB):
                emit_egroup(i0, 1, xt_sb, t_sb, ob)

            for p in pending:
                emit_pt(p)
            drain = list(pending); pending.clear()
            for p in drain:
                emit_ctx(p)
            s_ps = emit_scores(t_sb)
            pending.append(emit_softmax(i0, s_ps))

        for p in pending:
            emit_xn_dma(p)
        for p in pending:
            emit_pt(p)
        while pending:
            emit_ctx(pending.pop(0))
        nc.sync.dma_start(out=mz_out[:], in_=mz_all[0:1, :, :])

    nc.compile()
    return nc


def kernel(encoder_out, hidden, W_attn, b_attn, w_v, b_v, lengths):
    encoder_out = np.asarray(encoder_out)
    hidden = np.asarray(hidden)
    W_attn = np.asarray(W_attn)
    b_attn = np.asarray(b_attn)
    w_v = np.asarray(w_v)
    b_v = np.asarray(b_v)
    lengths = np.asarray(lengths)

    # ---- host-side work-unit schedule from the runtime lengths ----
    units = []  # (batch, s0, valid)
    for b in range(B):
        L = int(lengths[b])
        for s0 in range(0, L, CHUNK):
            units.append((b, s0, min(CHUNK, L - s0)))
    nchunk = max(1, (len(units) + N_CORES - 1) // N_CORES)

    W_e = W_attn[:, H:]                                    # [2H, 2H]
    # exact host-side per-batch bias and rank-1 score linearization
    C = hidden.T @ W_attn[:, :H].T + b_attn                # [B, 2H]
    u = W_e.T @ w_v[0]                                     # [2H]
    lin = encoder_out.reshape(-1, H2) @ u                  # [B*S]
    lin = lin.reshape(B, S)

    # ---- replicated weight layouts (fp8 DoubleRow), o-block-major ----
    # wet[ob, p, fb, q] = W_e^T[fb*128+p, ob*128+q] * SW
    wet = np.ascontiguousarray(
        W_e.T.reshape(FB, 128, OB, 128).transpose(2, 1, 0, 3) * SW
    ).astype(NP8)
    wv2 = np.ascontiguousarray(w_v[0].reshape(OB, 128).T).astype(np.float16)
    # the noise-correction functional wv.z~ = v.X8 only involves the
    # QUANTIZED operands, both known host-side -> computed exactly here and
    # folded into the mask/base row (no device work at all)
    # wet[ob,p,fb,q] = W8^T[fb*128+p, ob*128+q]*SW
    W8T = wet.astype(np.float32).transpose(2, 1, 0, 3).reshape(H2, H2) / SW
    v = W8T @ w_v[0]                                       # [2H], exact fp32
    v2 = (v / SX).reshape(FB, 128).T.astype(np.float32)    # [128, FB]

    # ---- per-core gathered inputs ----
    in_maps = []
    slot_of = []  # per real unit: (core, slot)
    x16 = encoder_out.astype(np.float16)
    for c in range(N_CORES):
        cu = units[c * nchunk:(c + 1) * nchunk]
        xt = np.zeros((nchunk, 128, FB, CHUNK), NP8)
        xn = np.zeros((nchunk, 128, CHUNK // 128, H2), np.float16)
        mask = np.full((nchunk, CHUNK), NEG + float(b_v[0]), np.float32)
        cb = np.zeros((128, OB, nchunk), np.float32)
        for slot, (b, s0, v) in enumerate(cu):
            chunk = encoder_out[b, s0:s0 + v, :]                 # [v, 2048]
            xt[slot, :, :, :v] = (
                (chunk.T * SX).reshape(FB, 128, v).transpose(1, 0, 2).astype(NP8))
            # xn[slot, p, sb, d] = chunk[sb*128 + p, d]
            full = np.zeros((CHUNK, H2), np.float16)
            full[:v] = x16[b, s0:s0 + v, :]
            xn[slot] = full.reshape(CHUNK // 128, 128, H2).transpose(1, 0, 2)
            corr = np.tensordot(xt[slot].astype(np.float32), v2,
                                axes=([0, 1], [0, 1]))       # wv . z~ exact
            mask[slot, :v] = (ALPHA * (lin[b, s0:s0 + v] - corr[:v])
                              + float(b_v[0]))
            cb[:, :, slot] = C[b].reshape(OB, 128).T
            slot_of.append((c, slot))
        in_maps.append(dict(
            xt=xt, xn=xn, mask=mask, cb=cb,
            wet=wet, wv=wv2,
        ))

    nc = build_program(nchunk)

    def run_once():
        res = run_bass_kernel_spmd(nc, in_maps, core_ids=list(range(N_CORES)))
        negm = np.stack([res.results[c]["out_mz"][:, 0] for c in range(N_CORES)])
        zz = np.stack([res.results[c]["out_mz"][:, 1] for c in range(N_CORES)])
        ctx = np.stack([res.results[c]["out_ctx"] for c in range(N_CORES)])
        return negm, zz, ctx

    def merge(parts):
        negm, zz, ctx = parts
        # ---- exact flash-softmax merge on host ----
        out = np.zeros((B, H2), np.float32)
        ok = np.isfinite(negm).all() and np.isfinite(zz).all() and np.isfinite(ctx).all()
        for b in range(B):
            idxs = [slot_of[k] for k, (ub, _, _) in enumerate(units) if ub == b]
            ms = np.array([-float(negm[c, s]) for c, s in idxs])
            m = ms.max()
            w = np.exp(ms - m)
            Z = float(sum(wi * float(zz[c, s]) for wi, (c, s) in zip(w, idxs)))
            if not (Z > 0):
                ok = False
                Z = 1.0
            acc = np.zeros(H2, np.float64)
            for wi, (c, s) in zip(w, idxs):
                acc += wi * ctx[c, s].astype(np.float64)
            out[b] = (acc / Z).astype(np.float32)
        # context rows are convex combinations of encoder_out rows
        ok = ok and np.isfinite(out).all() and np.abs(out).max() < 50.0
        return out, ok

    out, ok = merge(run_once())
    if not ok:  # one retry on gross corruption
        out, ok = merge(run_once())
    return out



# revision 50
# speedup vs baseline: 1.1991x; 1.1991x over previous
"""Bass/Trainium2 kernel for nn_Attention (ragged masked-softmax attention).

Math (per batch b with valid length L):
    c_b      = W_h @ hidden[:, b] + b_attn                  # [2H], W_h = W_attn[:, :H]
    e[s, :]  = tanh(W_e @ x_s + c_b)                        # W_e = W_attn[:, H:]
    score[s] = w_v . e[s, :] + b_v            (s < L)
    energy   = softmax(score[:L]);  context = energy @ X[:L]

Device strategy: the ragged work is split into fixed 256-position chunks
("units", 72 total for the graded lengths), distributed evenly over 8 cores
(one identical static SPMD program).  Each unit produces flash-softmax
partials (m, Z, ctx) which the host merges exactly.

The dominant e-matmul runs in fp8e4m3 with DoubleRow perf mode (2x the fp16
PE rate).  The fp8 quantization noise n on z = W_e x feeds the scores as
wv.(tanh'(z+c) (.) n); it is suppressed by an exact rank-1 correction
    score = wv.t~  -  a*(wv.z~ - wv.z),      a ~= E[tanh'] = 0.6
where BOTH linear functionals are computed exactly on the host (free):
wv.z = (W_e^T wv).x over the exact inputs, and wv.z~ = (W8^T wv).X8 over
the quantized operands the PE actually multiplies.  Their difference is
folded into the per-position mask/base row, so the device runs nothing
but the pure fp8 matmul + tanh + scores + softmax + context.  The exp
skips the max-subtraction pass (scores are provably bounded, so fp32
exp cannot overflow; the host flash-merge uses m=0).  Measured rel-err
~1.0e-2 against the fp32 reference (gate 2e-2).
"""

import numpy as np
import ml_dtypes

import concourse.bass as bass
import concourse.mybir as mybir
import concourse.tile as tile
from concourse import bacc
from concourse.bass_utils import run_bass_kernel_spmd

B, S, H = 16, 2048, 1024
H2 = 2 * H            # 2048 output features / encoder dim
CHUNK = 256           # sequence positions per work unit
N_CORES = 8
FB = H2 // 128        # 16 f-blocks of the contraction dim (encoder features)
OB = H2 // 128        # 16 o-blocks of the output features
NEG = -30000.0        # masked-score offset (exp underflows to exactly 0)
ALPHA = 0.6           # tanh'-projection coefficient of the fp8 correction
SW = 256.0            # fp8 scale on W_e
SX = 16.0             # fp8 scale on X
SV = 1024.0           # fp8 scale on the correction rows v = W8^T wv
INV_SWSX = 1.0 / (SW * SX)
CH = -ALPHA / (SV * SX)   # fold scale of the correction rows

F8 = mybir.dt.float8e4
F16 = mybir.dt.float16
F32 = mybir.dt.float32
NP8 = ml_dtypes.float8_e4m3


def build_program(nchunk: int):
    nc = bacc.Bacc()

    xt_ext = nc.declare_dram_parameter("xt", [nchunk, 128, FB, CHUNK], F8, isOutput=False)
    xn_ext = nc.declare_dram_parameter("xn", [nchunk, 128, CHUNK // 128, H2], F16, isOutput=False)
    mask_ext = nc.declare_dram_parameter("mask", [nchunk, CHUNK], F32, isOutput=False)
    c_ext = nc.declare_dram_parameter("cb", [128, OB, nchunk], F32, isOutput=False)
    wet_ext = nc.declare_dram_parameter("wet", [OB, 128, FB, 128], F8, isOutput=False)
    wv_ext = nc.declare_dram_parameter("wv", [128, OB], F16, isOutput=False)
    ctx_out = nc.declare_dram_parameter("out_ctx", [nchunk, H2], F32, isOutput=True)
    mz_out = nc.declare_dram_parameter("out_mz", [nchunk, 2], F32, isOutput=True)

    SB = CHUNK // 128   # s-blocks per unit for the context matmul
    DQ = H2 // 512      # 512-wide output quarters for the context matmul

    from contextlib import ExitStack
    with tile.TileContext(nc) as tc, ExitStack() as stk:
        singles = stk.enter_context(tc.tile_pool(name="singles", bufs=1))
        xtp = stk.enter_context(tc.tile_pool(name="xtp", bufs=3))
        xnp = stk.enter_context(tc.tile_pool(name="xnp", bufs=3))
        tp = stk.enter_context(tc.tile_pool(name="tp", bufs=2))
        smalls = stk.enter_context(tc.tile_pool(name="smalls", bufs=3))
        eps = stk.enter_context(tc.tile_pool(name="eps", bufs=4, space="PSUM"))
        sps = stk.enter_context(tc.tile_pool(name="sps", bufs=2, space="PSUM"))
        cps = stk.enter_context(tc.tile_pool(name="cps", bufs=1, space="PSUM"))

        # resident weights as one tile per o-block (fine-grained DMA deps so
        # the PE can start as soon as the first o-block's weights land)
        wet_sb = []
        wv_sb = singles.tile([128, OB], F16)
        c_sb = singles.tile([128, OB, nchunk], F32)
        mask_sb = singles.tile([1, nchunk, CHUNK], F32)
        xt0_sb = xtp.tile([128, FB, CHUNK], F8, tag="xt")
        xt1_sb = None
        if nchunk >= 2:
            xt1_sb = xtp.tile([128, FB, CHUNK], F8, tag="xt", name="xt1_sb")
        for ob in range(OB):
            if ob == 0:
                nc.sync.dma_start(out=xt0_sb[:], in_=xt_ext[0])
            if ob == 1 and xt1_sb is not None:
                nc.sync.dma_start(out=xt1_sb[:], in_=xt_ext[1])
            w1 = singles.tile([128, FB, 128], F8, tag=f"wet{ob}")
            nc.sync.dma_start(out=w1[:], in_=wet_ext[ob])
            wet_sb.append(w1)
            if ob == 0:
                nc.sync.dma_start(out=c_sb[:], in_=c_ext[:])
                nc.sync.dma_start(out=wv_sb[:], in_=wv_ext[:])
                nc.sync.dma_start(out=mask_sb[0:1, :, :], in_=mask_ext[:])
        mz_all = singles.tile([1, nchunk, 2], F32)
        nc.vector.memset(mz_all[:], 0.0)
        ident_sb = singles.tile([1, 1], F16)
        nc.vector.memset(ident_sb[:], 1.0)

        def emit_xn_dma(p):
            i, xn_sb = p[0], p[2]
            nc.sync.dma_start(out=xn_sb[:], in_=xn_ext[i])

        def emit_ctx(p):
            # 4 output quarters on the 4 PE column groups, running concurrently
            i, pt_sb, xn_sb = p[0], p[1], p[2]
            ctx_sb = smalls.tile([1, H2], F32, tag="ctx")
            ctx_ps = cps.tile([128, 512], F32, tag="cps")
            for dq in range(DQ):
                for sb in range(SB):
                    nc.tensor.matmul(
                        ctx_ps[32 * dq:32 * dq + 1, :],
                        lhsT=pt_sb[:, sb:sb + 1],
                        rhs=xn_sb[:, sb, dq * 512:(dq + 1) * 512],
                        start=(sb == 0), stop=(sb == SB - 1),
                        tile_position=(0, 32 * dq),
                    )
            for dq in range(DQ):
                nc.scalar.copy(
                    out=ctx_sb[0:1, dq * 512:(dq + 1) * 512],
                    in_=ctx_ps[32 * dq:32 * dq + 1, :])
            nc.sync.dma_start(out=ctx_out[i], in_=ctx_sb[0:1, :])

        def emit_egroup(i0, nw, xt_sb, t_sb, ob):
            e_ps = eps.tile([128, CHUNK], F32, tag="e")
            for fb in range(0, FB, 2):
                nc.tensor.matmul(
                    e_ps[:],
                    lhsT=wet_sb[ob][:, fb:fb + 2, :],
                    rhs=xt_sb[:, fb:fb + 2, :],
                    start=(fb == 0), stop=(fb == FB - 2),
                    perf_mode=mybir.MatmulPerfMode.DoubleRow,
                )
            nc.scalar.activation(
                out=t_sb[:, ob, :], in_=e_ps[:],
                func=mybir.ActivationFunctionType.Tanh,
                bias=c_sb[:, ob, i0:i0 + 1], scale=INV_SWSX,
            )

        def emit_scores(t_sb):
            # scores[s] = sum_o wv[o] t[o, s] -> 4 partial rows on the 4 PE
            # column groups running concurrently
            s_ps = sps.tile([128, CHUNK], F32, tag="st", bufs=1)
            for r in range(OB // 4):
                for j in range(4):
                    ob = r * 4 + j
                    nc.tensor.matmul(
                        s_ps[32 * j:32 * j + 1, :],
                        lhsT=wv_sb[:, ob:ob + 1],
                        rhs=t_sb[:, ob, :],
                        start=(r == 0), stop=(r == OB // 4 - 1),
                        tile_position=(0, 32 * j),
                    )
            return s_ps

        def emit_softmax(i, s_ps):
            # masked softmax partials: fold the 4 score rows + mask/base row
            # (DVE may read at most one PSUM operand per op -> serial chain)
            acc_sb = []
            srcs = [s_ps[32 * j:32 * j + 1, :] for j in range(4)]
            for j, src in enumerate(srcs):
                prev = mask_sb[0:1, i, :] if j == 0 else acc_sb[-1][:]
                a = smalls.tile([1, CHUNK], F32, tag=f"fold{j}")
                nc.vector.tensor_tensor(
                    out=a[:], in0=src, in1=prev,
                    op=mybir.AluOpType.add,
                )
                acc_sb.append(a)
            sc_sb = acc_sb[-1]
            # scores are bounded (|s| < ~10) -> exp never overflows fp32, so
            # the max-subtraction pass is skipped entirely (host merge m=0)
            p_sb = smalls.tile([1, CHUNK], F16, tag="p")
            z_sb = smalls.tile([1, 1], F32, tag="z")
            nc.scalar.activation(
                out=p_sb[:], in_=sc_sb[:],
                func=mybir.ActivationFunctionType.Exp,
                bias=0.0, scale=1.0, accum_out=z_sb[:],
            )
            nc.gpsimd.tensor_copy(out=mz_all[0:1, i, 1:2], in_=z_sb[:])
            xn_sb = xnp.tile([128, SB, H2], F16, tag="xn")
            return [i, p_sb, xn_sb]

        def emit_pt(p):
            # p row -> column layout [128, SB] via PE transpose.  Deferred to
            # the NEXT unit's PE stream (after its e-groups) so the transpose
            # never waits on the softmax chain.
            i, p_sb, xn_sb = p
            pt_sb = smalls.tile([128, SB], F16, tag="pt")
            for sb in range(SB):
                t_ps = sps.tile([128, 1], F16, tag="tp", bufs=2)
                nc.tensor.transpose(
                    t_ps[:], p_sb[0:1, sb * 128:(sb + 1) * 128], ident_sb[:])
                nc.vector.tensor_copy(out=pt_sb[:, sb:sb + 1], in_=t_ps[:])
            p[1] = pt_sb

        pending = []
        if nchunk >= 2:
            # ob-interleaved warm-up of units 0+1: each weight tile arriving
            # on the ramping DMA feeds TWO e-groups, so the PE stays busy
            # while the weight stream is still behind
            t0_sb = tp.tile([128, OB, CHUNK], F16, tag="t")
            t1_sb = tp.tile([128, OB, CHUNK], F16, tag="t")
            for ob in range(4):
                emit_egroup(0, 1, xt0_sb, t0_sb, ob)
            for ob in range(4, OB):
                emit_egroup(1, 1, xt1_sb, t1_sb, ob - 4)
                emit_egroup(0, 1, xt0_sb, t0_sb, ob)
            for ob in range(OB - 4, OB):
                emit_egroup(1, 1, xt1_sb, t1_sb, ob)
            s_ps = emit_scores(t0_sb)
            pending.append(emit_softmax(0, s_ps))
            s_ps = emit_scores(t1_sb)
            pending.append(emit_softmax(1, s_ps))
            start_i = 2
        else:
            start_i = 0
        for i0 in range(start_i, nchunk):
            if i0 == 0:
                xt_sb = xt0_sb
            else:
                xt_sb = xtp.tile([128, FB, CHUNK], F8, tag="xt")
                nc.sync.dma_start(out=xt_sb[:], in_=xt_ext[i0])
            for p in pending:
                emit_xn_dma(p)  # queued behind this unit's xt

            t_sb = tp.tile([128, OB, CHUNK], F16, tag="t")
            for ob in range(OB):
                emit_egroup(i0, 1, xt_sb, t_sb, ob)

            for p in pending:
                emit_pt(p)
            drain = list(pending); pending.clear()
            for p in drain:
                emit_ctx(p)
            s_ps = emit_scores(t_sb)
            pending.append(emit_softmax(i0, s_ps))

        for p in pending:
            emit_xn_dma(p)
        for p in pending:
            emit_pt(p)
        while pending:
            emit_ctx(pending.pop(0))
        nc.sync.dma_start(out=mz_out[:], in_=mz_all[0:1, :, :])

    nc.compile()
    return nc


def kernel(encoder_out, hidden, W_attn, b_attn, w_v, b_v, lengths):
    encoder_out = np.asarray(encoder_out)
    hidden = np.asarray(hidden)
    W_attn = np.asarray(W_attn)
    b_attn = np.asarray(b_attn)
    w_v = np.asarray(w_v)
    b_v = np.asarray(b_v)
    lengths = np.asarray(lengths)

    # ---- host-side work-unit schedule from the runtime lengths ----
    units = []  # (batch, s0, valid)
    for b in range(B):
        L = int(lengths[b])
        for s0 in range(0, L, CHUNK):
            units.append((b, s0, min(CHUNK, L - s0)))
    nchunk = max(1, (len(units) + N_CORES - 1) // N_CORES)

    W_e = W_attn[:, H:]                                    # [2H, 2H]
    # exact host-side per-batch bias and rank-1 score linearization
    C = hidden.T @ W_attn[:, :H].T + b_attn                # [B, 2H]
    u = W_e.T @ w_v[0]                                     # [2H]
    lin = encoder_out.reshape(-1, H2) @ u                  # [B*S]
    lin = lin.reshape(B, S)

    # ---- replicated weight layouts (fp8 DoubleRow), o-block-major ----
    # wet[ob, p, fb, q] = W_e^T[fb*128+p, ob*128+q] * SW
    wet = np.ascontiguousarray(
        W_e.T.reshape(FB, 128, OB, 128).transpose(2, 1, 0, 3) * SW
    ).astype(NP8)
    wv2 = np.ascontiguousarray(w_v[0].reshape(OB, 128).T).astype(np.float16)
    # the noise-correction functional wv.z~ = v.X8 only involves the
    # QUANTIZED operands, both known host-side -> computed exactly here and
    # folded into the mask/base row (no device work at all)
    # wet[ob,p,fb,q] = W8^T[fb*128+p, ob*128+q]*SW
    W8T = wet.astype(np.float32).transpose(2, 1, 0, 3).reshape(H2, H2) / SW
    v = W8T @ w_v[0]                                       # [2H], exact fp32
    v2 = (v / SX).reshape(FB, 128).T.astype(np.float32)    # [128, FB]

    # ---- per-core gathered inputs ----
    in_maps = []
    slot_of = []  # per real unit: (core, slot)
    x16 = encoder_out.astype(np.float16)
    for c in range(N_CORES):
        cu = units[c * nchunk:(c + 1) * nchunk]
        xt = np.zeros((nchunk, 128, FB, CHUNK), NP8)
        xn = np.zeros((nchunk, 128, CHUNK // 128, H2), np.float16)
        mask = np.full((nchunk, CHUNK), NEG + float(b_v[0]), np.float32)
        cb = np.zeros((128, OB, nchunk), np.float32)
        for slot, (b, s0, v) in enumerate(cu):
            chunk = encoder_out[b, s0:s0 + v, :]                 # [v, 2048]
            xt[slot, :, :, :v] = (
                (chunk.T * SX).reshape(FB, 128, v).transpose(1, 0, 2).astype(NP8))
            # xn[slot, p, sb, d] = chunk[sb*128 + p, d]
            full = np.zeros((CHUNK, H2), np.float16)
            full[:v] = x16[b, s0:s0 + v, :]
            xn[slot] = full.reshape(CHUNK // 128, 128, H2).transpose(1, 0, 2)
            corr = np.tensordot(xt[slot].astype(np.float32), v2,
                                axes=([0, 1], [0, 1]))       # wv . z~ exact
            mask[slot, :v] = (ALPHA * (lin[b, s0:s0 + v] - corr[:v])
                              + float(b_v[0]))
            cb[:, :, slot] = C[b].reshape(OB, 128).T
            slot_of.append((c, slot))
        in_maps.append(dict(
            xt=xt, xn=xn, mask=mask, cb=cb,
            wet=wet, wv=wv2,
        ))

    nc = build_program(nchunk)

    def run_once():
        res = run_bass_kernel_spmd(nc, in_maps, core_ids=list(range(N_CORES)))
        negm = np.stack([res.results[c]["out_mz"][:, 0] for c in range(N_CORES)])
        zz = np.stack([res.results[c]["out_mz"][:, 1] for c in range(N_CORES)])
        ctx = np.stack([res.results[c]["out_ctx"] for c in range(N_CORES)])
        return negm, zz, ctx

    def merge(parts):
        negm, zz, ctx = parts
        # ---- exact flash-softmax merge on host ----
        out = np.zeros((B, H2), np.float32)
        ok = np.isfinite(negm).all() and np.isfinite(zz).all() and np.isfinite(ctx).all()
        for b in range(B):
            idxs = [slot_of[k] for k, (ub, _, _) in enumerate(units) if ub == b]
            ms = np.array([-float(negm[c, s]) for c, s in idxs])
            m = ms.max()
            w = np.exp(ms - m)
            Z = float(sum(wi * float(zz[c, s]) for wi, (c, s) in zip(w, idxs)))
            if not (Z > 0):
                ok = False
                Z = 1.0
            acc = np.zeros(H2, np.float64)
            for wi, (c, s) in zip(w, idxs):
                acc += wi * ctx[c, s].astype(np.float64)
            out[b] = (acc / Z).astype(np.float32)
        # context rows are convex combinations of encoder_out rows
        ok = ok and np.isfinite(out).all() and np.abs(out).max() < 50.0
        return out, ok

    out, ok = merge(run_once())
    if not ok:  # one retry on gross corruption
        out, ok = merge(run_once())
    return out

